# revision 2
# baseline (speedup 1.0000x reference)
"""Trainium2 Bass kernel for DDGAttention (B=4, L=2048, D=256, H=8, DQK=DV=32).

Sharding: 8 cores = 4 batches x 2 query-halves; each core handles 1024 queries
against all 2048 keys of its batch (keys rolled per core so local queries are
rows 0..1023).

Phase A pipeline (ACT-bound by design): a 4-slot PSUM ring holds QK logits
[128 keys, 512 q] per (head, key-block) unit; units stream as (g, hc, kc, h')
with 2-way row-tiled QK half-waves; one exp [128,1024] per half-wave feeds a
24-slot SBUF ex ring; AV consumes ex with 4-way col-tiled matmuls into
av_v (v-features, M=32/head) and av_p (pcb+1, M=4/head). The ring phasing
(window 2 of ring 4) keeps ACT exp back-to-back with zero bubbles.

Softmax rows sum to 1, so atom_pos_bias = alpha @ pos_CB - pos_CA and the
denominator comes from the ones column of av_p. exp needs no running max
(logits are O(3) by construction). sqrt/rsqrt are exp(+-0.5*ln(x)) so all ACT
ops live in the natural_log_exp table set.
"""
import sys

sys.path.insert(0, "/opt/trn_rl_repo")

import numpy as np

B, L, DIN, DOUT = 4, 2048, 256, 256
H, DQK, DV = 8, 32, 32
NQ = L // 2          # queries per core
LB = L // 128        # 16 key blocks

_cache = {}


def _build(phases=('pro', 'A', 'B'), rep=1, debug_outs=False):
    import concourse.mybir as mybir
    import concourse.tile as tile
    from concourse import bacc, library_config
    from concourse.masks import make_identity

    F32 = mybir.dt.float32
    BF16 = mybir.dt.bfloat16
    F32R = mybir.dt.float32r
    AF = mybir.ActivationFunctionType
    ALU = mybir.AluOpType
    AX = mybir.AxisListType

    import concourse.bacc as bacc_mod
    real_tables = bacc_mod.get_activation_tables("gen3")
    patched = {}
    for name, funcs in real_tables.items():
        funcs = set(funcs)
        if name != "natural_log_exp_and_others":
            funcs.discard(mybir.ActivationFunctionType.Exp)
            funcs.discard(mybir.ActivationFunctionType.Ln)
        patched[name] = funcs
    bacc_mod.get_activation_tables = lambda arch, _p=patched: _p

    nc = bacc.Bacc("TRN2", target_bir_lowering=False, debug=False, num_devices=8)

    x_d = nc.dram_tensor("x", [L, DIN], F32, kind="ExternalInput")
    pcb_d = nc.dram_tensor("pcb", [L, 3], F32, kind="ExternalInput")
    pca_d = nc.dram_tensor("pca", [NQ, 3], F32, kind="ExternalInput")
    frm_d = nc.dram_tensor("frm", [NQ, 9], F32, kind="ExternalInput")
    wq_d = nc.dram_tensor("wq", [DIN, 256], F32R, kind="ExternalInput")
    wk_d = nc.dram_tensor("wk", [DIN, 256], F32R, kind="ExternalInput")
    wv_d = nc.dram_tensor("wv", [DIN, 256], F32R, kind="ExternalInput")
    won_d = nc.dram_tensor("won", [32, 8 * 256], BF16, kind="ExternalInput")
    wos_d = nc.dram_tensor("wos", [57, 256], BF16, kind="ExternalInput")
    g_d = nc.dram_tensor("g", [1, 256], F32, kind="ExternalInput")
    bb_d = nc.dram_tensor("bb", [1, 256], F32, kind="ExternalInput")
    out_d = nc.dram_tensor("out", [NQ, DOUT], F32, kind="ExternalOutput")
    if debug_outs:
        dbg_d = {
            "dfT0": nc.dram_tensor("dfT0", [128, NQ], BF16, kind="ExternalOutput"),
            "dfT1": nc.dram_tensor("dfT1", [128, NQ], BF16, kind="ExternalOutput"),
            "dwp0": nc.dram_tensor("dwp0", [128, NQ], F32, kind="ExternalOutput"),
            "dwp1": nc.dram_tensor("dwp1", [128, NQ], F32, kind="ExternalOutput"),
            "dfn": nc.dram_tensor("dfn", [32, H * NQ], BF16, kind="ExternalOutput"),
            "dex": nc.dram_tensor("dex", [128, 24 * 512], BF16, kind="ExternalOutput"),
        }

    with tile.TileContext(nc) as tc, tc.tile_pool(name="per", bufs=1) as per:
        nc.gpsimd.load_library(library_config.attn)

        ident = per.tile([128, 128], F32)
        make_identity(nc, ident[:])

        # persistent SBUF tensors
        x_sb = per.tile([128, LB * 256], F32)     # x[128b+p, d] -> [p, b*256+d]
        xT_sb = per.tile([128, 2 * L], F32R)      # xT[p, c*L+l] = x[l, 128c+p]
        kT_sb = per.tile([128, 2 * L], F32R)      # kT[p, c*L+l] = k[l, 128c+p]
        qT_sb = per.tile([128, 2 * NQ], F32R)
        v2_sb = per.tile([128, LB, 8, 36], BF16)  # [v(32) | 1 | pcb(3)] per (kc, h)
        ex_ring = per.tile([128, 24 * 512], BF16)
        wq_sb = per.tile([128, 2 * 256], F32R)
        wk_sb = per.tile([128, 2 * 256], F32R)
        wv_sb = per.tile([128, 2 * 256], F32R)
        won_sb = per.tile([32, 8 * 256], BF16)    # [d, h*256+o] = Wo[32h+d, o]
        wos_sb = per.tile([57, 256], BF16)        # rows: 56 spatial + bias row
        gg_sb = per.tile([128, 256], F32)
        bb_sb = per.tile([128, 256], F32)
        gg16 = per.tile([128, 256], BF16)
        bb16 = per.tile([128, 256], BF16)
        # per-(quad, pair) feature tensors: even head at rows 0-35,
        # odd head at rows 64-99 (v | 1 | pcb layout along rows)
        fTp = [[per.tile([128, NQ], BF16, name=f"fT{q}{p}") for p in range(2)]
               for q in range(2)]
        wpTp = [[per.tile([128, NQ], F32, name=f"wp{q}{p}") for p in range(2)]
                for q in range(2)]
        fnT_all = per.tile([32, H, NQ], BF16, name="fnT_all")
        # zero-padded AV bracket weights: full-128-col lhsT for the single
        # start and stop matmul of each av PSUM bank
        v0pad = [[per.tile([128, 128], BF16, name=f"v0pad{q}{p}")
                  for p in range(2)] for q in range(2)]
        vLpad = [[per.tile([128, 128], BF16, name=f"vLpad{q}{p}")
                  for p in range(2)] for q in range(2)]
        b30 = per.tile([128, 1], F32)
        b5 = per.tile([128, 1], F32)
        c256 = per.tile([128, 1], F32)
        c10 = per.tile([128, 1], F32)
        nc.vector.memset(b30[:], 1e-30)
        nc.vector.memset(b5[:], 1e-5)
        nc.vector.memset(c256[:], 1.0 / 256)
        nc.vector.memset(c10[:], 1e-10)

        # input loads
        for dk in range(8):
            nc.sync.dma_start(
                x_sb[:, dk * 512 : dk * 512 + 512].rearrange(
                    "p (b d) -> p b d", d=256
                ),
                x_d.rearrange("(b p) d -> p b d", p=128)[:, 2 * dk : 2 * dk + 2, :],
            )
        for w_sb, w_d in ((wq_sb, wq_d), (wk_sb, wk_d), (wv_sb, wv_d)):
            nc.sync.dma_start(
                w_sb[:].rearrange("p (c d) -> p c d", d=256),
                w_d.rearrange("(c p) d -> p c d", p=128),
            )
        nc.sync.dma_start(won_sb[:], won_d[:])
        nc.sync.dma_start(wos_sb[:], wos_d[:])
        g1 = per.tile([1, 256], F32)
        b1 = per.tile([1, 256], F32)
        nc.sync.dma_start(g1[:], g_d[:])
        nc.sync.dma_start(b1[:], bb_d[:])
        nc.gpsimd.partition_broadcast(gg_sb[:], g1[:])
        nc.gpsimd.partition_broadcast(bb_sb[:], b1[:])
        nc.vector.tensor_copy(gg16[:], gg_sb[:])
        nc.vector.tensor_copy(bb16[:], bb_sb[:])
        pcb_f = per.tile([128, LB, 3], F32)
        nc.sync.dma_start(
            pcb_f[:], pcb_d.rearrange("(b p) d -> p b d", p=128)
        )

        # ---------- prologue: xT, kT, qT, v ----------
        with tc.tile_pool(name="pro_ps", bufs=2, space="PSUM") as pro_ps:
            for blk in range(LB):
                for c in range(2):
                    tps = pro_ps.tile([128, 128], F32, tag="tp")
                    nc.tensor.transpose(
                        tps[:],
                        x_sb[:, blk * 256 + c * 128 : blk * 256 + c * 128 + 128],
                        ident[:],
                    )
                    dst = xT_sb[:, c * L + blk * 128 : c * L + blk * 128 + 128]
                    if blk % 2 == 0:
                        nc.vector.tensor_copy(dst, tps[:])
                    else:
                        nc.scalar.copy(dst, tps[:])

            # kT[hd, l] = Wk.T @ xT ; lhsT = Wk chunk [din128, hd128]
            for hcc in range(2):
                for lg in range(L // 512):
                    kps = pro_ps.tile([128, 512], F32, tag="proj", bufs=4)
                    for dc in range(2):
                        nc.tensor.matmul(
                            kps[:],
                            wk_sb[:, dc * 256 + hcc * 128 : dc * 256 + hcc * 128 + 128],
                            xT_sb[:, dc * L + lg * 512 : dc * L + lg * 512 + 512],
                            start=(dc == 0),
                            stop=(dc == 1),
                        )
                    nc.scalar.copy(
                        kT_sb[:, hcc * L + lg * 512 : hcc * L + lg * 512 + 512],
                        kps[:],
                    )
            # qT (only local 1024 query columns)
            for hcc in range(2):
                for qg in range(NQ // 512):
                    qps = pro_ps.tile([128, 512], F32, tag="proj", bufs=4)
                    for dc in range(2):
                        nc.tensor.matmul(
                            qps[:],
                            wq_sb[:, dc * 256 + hcc * 128 : dc * 256 + hcc * 128 + 128],
                            xT_sb[:, dc * L + qg * 512 : dc * L + qg * 512 + 512],
                            start=(dc == 0),
                            stop=(dc == 1),
                        )
                    nc.scalar.copy(
                        qT_sb[:, hcc * NQ + qg * 512 : hcc * NQ + qg * 512 + 512],
                        qps[:],
                    )
            # v rows: v[l, hd]; lhsT = xT chunk [din128, l128]
            for blk in range(LB):
                vps = pro_ps.tile([128, 256], F32, tag="vproj")
                for dc in range(2):
                    nc.tensor.matmul(
                        vps[:],
                        xT_sb[:, dc * L + blk * 128 : dc * L + blk * 128 + 128],
                        wv_sb[:, dc * 256 : dc * 256 + 256],
                        start=(dc == 0),
                        stop=(dc == 1),
                    )
                v2b = v2_sb[:, blk, :, :]
                nc.vector.tensor_copy(
                    v2b[:, :, 0:32], vps[:].rearrange("p (h d) -> p h d", d=32)
                )
                nc.vector.memset(v2b[:, :, 32:33], 1.0)
                nc.vector.tensor_copy(
                    v2b[:, :, 33:36],
                    pcb_f[:, blk, None, :].broadcast_to([128, 8, 3]),
                )
            for q in range(2):
                for p2 in range(2):
                    he, ho = 4 * q + 2 * p2, 4 * q + 2 * p2 + 1
                    nc.vector.memset(v0pad[q][p2][:], 0.0)
                    nc.vector.memset(vLpad[q][p2][:], 0.0)
                    nc.vector.tensor_copy(
                        v0pad[q][p2][:, 0:36], v2_sb[:, 0, he, :]
                    )
                    nc.vector.tensor_copy(
                        vLpad[q][p2][:, 64:100], v2_sb[:, LB - 1, ho, :]
                    )

        # ---------- phase A: attention ----------
        for _rep in range(rep):
         with tc.tile_pool(name="ring_ps", bufs=1, space="PSUM") as ring_ps, \
             tc.tile_pool(name="av_ps", bufs=1, space="PSUM") as av_ps, \
             tc.tile_pool(name="nrm_sb", bufs=2) as nrm_sb:
            st_ring = ring_ps.tile([128, 6 * 512], F32, name="st_ring")

            def av_wave(avp, g, hc, kc):
                base = 64 * (2 * g + hc) + 4 * kc
                # order: latest-exp first so one satisfied wait covers the wave
                if kc == 0:
                    order = [(0, 0, "start"), (1, 0, "start"), (0, 1, "mid"),
                             (1, 1, "mid")]
                elif kc == LB - 1:
                    order = [(0, 0, "mid"), (1, 0, "mid"), (0, 1, "stop"),
                             (1, 1, "stop")]
                else:
                    order = [(1, 1, "mid"), (1, 0, "mid"), (0, 1, "mid"),
                             (0, 0, "mid")]
                for p2, par, kind in order:
                    hp = 2 * p2 + par
                    es = ((base + hp) % 24) * 512
                    ex = ex_ring[:, es : es + 512]
                    if kind == "start":
                        nc.tensor.matmul(avp[p2][:, :], v0pad[hc][p2][:], ex,
                                         start=True, stop=False)
                    elif kind == "stop":
                        nc.tensor.matmul(avp[p2][:, :], vLpad[hc][p2][:], ex,
                                         start=False, stop=True)
                    else:
                        r0 = 64 * par
                        nc.tensor.matmul(
                            avp[p2][r0 : r0 + 36, :],
                            v2_sb[:, kc, 4 * hc + hp, :],
                            ex,
                            start=False, stop=False,
                            tile_position=(0, r0),
                        )

            AV_LAG = 3
            do_av = not ("A1" in phases)
            do_norm = not ("A1" in phases or "A2" in phases)
            u = 0  # global unit counter
            for g in (range(2) if ("A" in phases or "A1" in phases or "A2" in phases) else []):
                for hc in range(2):
                    avp = [av_ps.tile([128, 512], F32, tag=f"av{p2}",
                                      name=f"av{p2}") for p2 in range(2)]
                    for kc in range(LB):
                        for hp in range(4):
                            slot = (u + hp) % 6
                            nc.tensor.matmul(
                                st_ring[:, slot * 512 : slot * 512 + 512],
                                kT_sb[32 * hp : 32 * hp + 32,
                                      hc * L + kc * 128 : hc * L + kc * 128 + 128],
                                qT_sb[32 * hp : 32 * hp + 32,
                                      hc * NQ + g * 512 : hc * NQ + g * 512 + 512],
                                start=True,
                                stop=True,
                                tile_position=(32 * hp, 0),
                            )
                        for half in range(2):
                            w = ((u + 2 * half) // 2) % 12
                            s0 = (u + 2 * half) % 6
                            nc.scalar.activation(
                                ex_ring[:, w * 1024 : w * 1024 + 1024],
                                st_ring[:, s0 * 512 : s0 * 512 + 1024],
                                AF.Exp,
                            )
                        u += 4
                        # AV lags 3 key blocks so it never heads the PE queue
                        # and the quad-boundary norm chain stays off-path
                        if kc >= AV_LAG and do_av:
                            av_wave(avp, g, hc, kc - AV_LAG)
                    if do_av:
                        for kcf in range(LB - AV_LAG, LB):
                            av_wave(avp, g, hc, kcf)
                    if not do_norm:
                        continue
                    # normalization per pair: one reciprocal of the av tile
                    # (denominator rows 32 and 96), broadcast at partition 0,
                    # shift into place, then one TT for v-features and one
                    # for pos rows
                    for p2 in range(2):
                        rp = nrm_sb.tile([128, 512], F32, tag=f"rp{p2}",
                                         name=f"rp{p2}")
                        nc.vector.reciprocal(rp[:], avp[p2][:])
                        rb = nrm_sb.tile([128, 512], F32, tag=f"rb{p2}",
                                         name=f"rb{p2}")
                        for par in range(2):
                            r1 = nrm_sb.tile([1, 512], F32,
                                             tag=f"r1_{p2}{par}",
                                             name=f"r1_{p2}{par}")
                            nc.sync.dma_start(
                                r1[:], rp[64 * par + 32 : 64 * par + 33, :]
                            )
                            rbh = nrm_sb.tile([36, 512], F32,
                                              tag=f"rbh{p2}{par}",
                                              name=f"rbh{p2}{par}")
                            nc.gpsimd.partition_broadcast(rbh[:], r1[:])
                            if par == 0:
                                nc.vector.tensor_copy(rb[0:36, :], rbh[:])
                            else:
                                nc.sync.dma_start(rb[64:100, :], rbh[:])
                        nc.vector.tensor_tensor(
                            fTp[hc][p2][:, g * 512 : g * 512 + 512],
                            avp[p2][:], rb[:], op=ALU.mult,
                        )
                        nc.vector.tensor_tensor(
                            wpTp[hc][p2][:, g * 512 : g * 512 + 512],
                            avp[p2][:], rb[:], op=ALU.mult,
                        )
                    for hp in range(4):
                        p2, par = hp // 2, hp % 2
                        nc.sync.dma_start(
                            fnT_all[:, 4 * hc + hp, g * 512 : g * 512 + 512],
                            fTp[hc][p2][64 * par : 64 * par + 32,
                                        g * 512 : g * 512 + 512],
                        )

         if debug_outs:
            nc.sync.dma_start(dbg_d["dfT0"][:], fTp[0][0][:])
            nc.sync.dma_start(dbg_d["dfT1"][:], fTp[1][0][:])
            nc.sync.dma_start(dbg_d["dwp0"][:], wpTp[0][0][:])
            nc.sync.dma_start(dbg_d["dwp1"][:], wpTp[1][0][:])
            nc.sync.dma_start(
                dbg_d["dfn"][:], fnT_all[:].rearrange("p h q -> p (h q)")
            )
            nc.sync.dma_start(dbg_d["dex"][:], ex_ring[:])

        # ---------- phase B: spatial features + out proj + LN ----------
         with tc.tile_pool(name="tp_ps", bufs=2, space="PSUM") as tp_ps, \
              tc.tile_pool(name="op_ps", bufs=2, space="PSUM") as op_ps, \
              tc.tile_pool(name="bp", bufs=2) as bp:
            for gh in (range(2) if "B" in phases else []):
                # ---- batched spatial features for 4 query blocks ----
                wq_all = bp.tile([128, 4, 24], F32, tag="wq_all", name="wq_all")
                for qb4 in range(4):
                    q0 = (gh * 4 + qb4) * 128
                    wqps = tp_ps.tile([128, 4, 128], F32, tag="wq", name="wqps")
                    for t in range(4):
                        nc.tensor.transpose(
                            wqps[:, t, :],
                            wpTp[t // 2][t % 2][:, q0 : q0 + 128],
                            ident[:],
                        )
                    # pos cols at 33-35 (even head) / 97-99 (odd head) of each
                    # pair-tile -> head-major (t, par, j) = h*3+j ordering
                    nc.vector.tensor_copy(
                        wq_all[:, qb4, :].rearrange(
                            "p (t par j) -> p t par j", t=4, j=3
                        ),
                        wqps[:].rearrange(
                            "p t (par r) -> p t par r", r=64
                        )[:, :, :, 33:36],
                    )
                pca_t = bp.tile([128, 4, 3], F32, tag="pca", name="pca_t")
                nc.sync.dma_start(
                    pca_t[:],
                    pca_d.rearrange("(qb p) c -> p qb c", p=128)[
                        :, gh * 4 : gh * 4 + 4, :
                    ],
                )
                frm_t = bp.tile([128, 4, 9], F32, tag="frm", name="frm_t")
                nc.sync.dma_start(
                    frm_t[:],
                    frm_d.rearrange("(qb p) c -> p qb c", p=128)[
                        :, gh * 4 : gh * 4 + 4, :
                    ],
                )
                apb = bp.tile([128, 4, 8, 3], F32, tag="apb", name="apb")
                nc.vector.tensor_tensor(
                    apb,
                    wq_all.rearrange("p q (h j) -> p q h j", j=3),
                    pca_t[:, :, None, :].broadcast_to([128, 4, 8, 3]),
                    op=ALU.subtract,
                )
                fsp = bp.tile([128, 4, 57], F32, tag="fsp", name="fsp")
                fsp_p = fsp[:, :, 0:24].rearrange("p q (h i) -> p q h i", i=3)
                tmp = bp.tile([128, 4, 8, 3], F32, tag="tmp", name="tmp")
                for i in range(3):
                    nc.vector.tensor_tensor(
                        tmp, apb,
                        frm_t[:, :, 3 * i : 3 * i + 3][
                            :, :, None, :
                        ].broadcast_to([128, 4, 8, 3]),
                        op=ALU.mult,
                    )
                    nc.vector.tensor_reduce(
                        fsp_p[:, :, :, i], tmp, axis=AX.X, op=ALU.add
                    )
                # distance = sqrt(sum apb^2) = exp(.5 ln)
                sq = bp.tile([128, 4, 8, 3], F32, tag="sq", name="sq")
                nc.vector.tensor_tensor(sq, apb, apb, op=ALU.mult)
                d2 = bp.tile([128, 4, 8], F32, tag="d2", name="d2")
                nc.vector.tensor_reduce(d2[:], sq, axis=AX.X, op=ALU.add)
                nc.scalar.activation(d2[:], d2[:], AF.Ln, bias=b30[:])
                nc.scalar.activation(fsp[:, :, 24:32], d2[:], AF.Exp, scale=0.5)
                # direction = fp / (|fp| + 1e-10)
                nc.vector.tensor_tensor(sq, fsp_p, fsp_p, op=ALU.mult)
                f2 = bp.tile([128, 4, 8], F32, tag="f2", name="f2")
                nc.vector.tensor_reduce(f2[:], sq, axis=AX.X, op=ALU.add)
                nc.scalar.activation(f2[:], f2[:], AF.Ln, bias=b30[:])
                nc.scalar.activation(f2[:], f2[:], AF.Exp, scale=0.5)
                nc.vector.tensor_tensor(
                    f2[:], f2[:],
                    c10[:, None, :].broadcast_to([128, 4, 8]), op=ALU.add
                )
                nc.vector.reciprocal(f2[:], f2[:])
                nc.vector.tensor_tensor(
                    fsp[:, :, 32:56].rearrange("p q (h i) -> p q h i", i=3),
                    fsp_p,
                    f2[:, :, :, None].broadcast_to([128, 4, 8, 3]),
                    op=ALU.mult,
                )
                nc.vector.memset(fsp[:, :, 56:57], 1.0)
                # ---- per-block out-projection into batched y ----
                y = bp.tile([128, 4, 256], BF16, tag="y", name="y")
                for qb4 in range(4):
                    qb = gh * 4 + qb4
                    q0 = qb * 128
                    fspT_ps = tp_ps.tile([57, 128], F32, tag="fspT",
                                         name="fspT_ps")
                    nc.tensor.transpose(fspT_ps[:], fsp[:, qb4, :], ident[:])
                    fspT = bp.tile([57, 128], BF16, tag="fspTs", name="fspTs")
                    nc.scalar.copy(fspT[:], fspT_ps[:])
                    o_t = op_ps.tile([128, 256], F32, tag="o", name="o_t")
                    for h in range(H):
                        nc.tensor.matmul(
                            o_t[:],
                            fnT_all[:, h, q0 : q0 + 128],
                            won_sb[:, h * 256 : h * 256 + 256],
                            start=(h == 0),
                            stop=False,
                        )
                    nc.tensor.matmul(o_t[:], fspT[:], wos_sb[:],
                                     start=False, stop=True)
                    nc.vector.tensor_tensor(
                        y[:, qb4, :], o_t[:],
                        x_sb[:, qb * 256 : qb * 256 + 256], op=ALU.add
                    )
                # ---- batched residual layernorm over [128, 4, 256] ----
                m = bp.tile([128, 4], F32, tag="m", name="m")
                nc.vector.tensor_reduce(m[:], y, axis=AX.X, op=ALU.add)
                nc.vector.tensor_tensor(
                    m[:], m[:], c256[:].broadcast_to([128, 4]), op=ALU.mult
                )
                m16 = bp.tile([128, 4], BF16, tag="m16", name="m16")
                nc.vector.tensor_copy(m16[:], m[:])
                cent = bp.tile([128, 4, 256], BF16, tag="cent", name="cent")
                nc.vector.tensor_tensor(
                    cent, y, m16[:, :, None].broadcast_to([128, 4, 256]),
                    op=ALU.subtract,
                )
                sqs = bp.tile([128, 4, 256], BF16, tag="sqs", name="sqs")
                nc.vector.tensor_tensor(sqs, cent, cent, op=ALU.mult)
                var = bp.tile([128, 4], F32, tag="var", name="var")
                nc.vector.tensor_reduce(var[:], sqs, axis=AX.X, op=ALU.add)
                nc.scalar.activation(var[:], var[:], AF.Ln, bias=b5[:],
                                     scale=1.0 / 256)
                nc.scalar.activation(var[:], var[:], AF.Exp, scale=-0.5)
                rstd = bp.tile([128, 4], BF16, tag="rstd", name="rstd")
                nc.vector.tensor_copy(rstd[:], var[:])
                ob = bp.tile([128, 4, 256], BF16, tag="ob", name="ob")
                nc.vector.tensor_tensor(
                    ob, cent, rstd[:, :, None].broadcast_to([128, 4, 256]),
                    op=ALU.mult,
                )
                nc.vector.tensor_tensor(
                    ob, ob, gg16[:, None, :].broadcast_to([128, 4, 256]),
                    op=ALU.mult,
                )
                obf = bp.tile([128, 4, 256], F32, tag="obf", name="obf")
                nc.vector.tensor_tensor(
                    obf, ob, bb16[:, None, :].broadcast_to([128, 4, 256]),
                    op=ALU.add,
                )
                nc.sync.dma_start(
                    out_d.rearrange("(qb p) d -> p qb d", p=128)[
                        :, gh * 4 : gh * 4 + 4, :
                    ],
                    obf[:],
                )

    nc.compile()
    return nc


def _prep_inputs(x, pos_CA, pos_CB, frame, Wq, Wk, Wv, Wo, bo, ln_g, ln_b):
    import ml_dtypes
    bf16 = ml_dtypes.bfloat16
    won = (
        np.ascontiguousarray(Wo[:256].reshape(8, 32, 256).transpose(1, 0, 2))
        .reshape(32, 8 * 256)
        .astype(bf16)
    )
    wos = np.concatenate([Wo[256:312], bo[None, :]], axis=0).astype(bf16)
    maps = []
    for core in range(8):
        b, qh = core // 2, core % 2
        roll = -qh * NQ
        maps.append(
            {
                "x": np.ascontiguousarray(np.roll(x[b], roll, axis=0)),
                "pcb": np.ascontiguousarray(np.roll(pos_CB[b], roll, axis=0)),
                "pca": np.ascontiguousarray(pos_CA[b, qh * NQ : (qh + 1) * NQ]),
                "frm": np.ascontiguousarray(
                    frame[b, qh * NQ : (qh + 1) * NQ].reshape(NQ, 9)
                ),
                "wq": Wq,
                "wk": Wk,
                "wv": Wv,
                "won": won,
                "wos": wos,
                "g": ln_g[None, :],
                "bb": ln_b[None, :],
            }
        )
    return maps


def kernel(**inputs):
    from concourse.bass_utils import run_bass_kernel_spmd

    inputs = {k: np.asarray(v, dtype=np.float32) for k, v in inputs.items()}
    if "nc" not in _cache:
        _cache["nc"] = _build()
    nc = _cache["nc"]
    in_maps = _prep_inputs(**inputs)
    res = run_bass_kernel_spmd(nc, in_maps, list(range(8)))
    out = np.empty((B, L, DOUT), dtype=np.float32)
    for core in range(8):
        b, qh = core // 2, core % 2
        out[b, qh * NQ : (qh + 1) * NQ] = res.results[core]["out"]
    return out
